# revision 1
# baseline (speedup 1.0000x reference)
"""MiniBatchDiscrimination kernel for 8 Trainium2 NeuronCores.

Problem:
  x [256, 1024] f32, T [1024, 128, 16] f32
  M = einsum('na,abc->nbc', x, T)                      [N=256, B=128, C=16]
  D[k,j,b] = sum_c |M[k,b,c] - M[j,b,c]|
  Cmat = exp(-D); S = sum_j Cmat
  out = S - Cmat[:, N-1, :]; out[0] = S[0]-Cmat[0,0]; out[N-1] = S[N-1]-Cmat[N-1,N-1]

Sharding: data-parallel over B (each core owns 16 of the 128 b-channels).
The pairwise distance is independent per b, so there is no communication.

Per-core dataflow (abs decomposed as |d| = 2*relu(d) - d, since abs_max is
not a valid TRN2 DVE ALU op but relu (sub,max,0) is a single 2x-mode op;
the linear term sum_c d = R[b,j] - R[b,k] is folded in by PE and the exp
bias). Everything streams in fp16 except the f32 PSUM accumulations:
  PE   : MT[bc, n] = (x @ T_loc)^T via 16 accumulating matmuls (a-chunks)
  PE   : R[b, j] = sum_c M[j, b, c]  (pattern matmul)
  DVE  : per k: relu(MT[:, j] - MT[:, k]) via tensor_scalar(sub, max, 0),
         4x perf mode; 1 of 8 k's runs on ScalarE activation(Relu) instead,
         emitted one group ahead so ScalarE's FIFO can't stall PE
  PE   : c-reduction: 2*pattern^T @ relu-tile -> 2P, 4 k's per [128, 256]
         PSUM bank via col-group tile_position (16-row slices at 32g); one
         fold matmul per bank adds -R[b, j]
  ScE  : exp(-psD + bias), bias = -R[b,k] per partition; accum_out emits
         the row sums S (the j-reduction) for free
  Pool : extract Cmat[:, 255] columns (and Cmat[0,0])
  DVE  : final out = S - C255 (+ k=0 self fix), in two halves to overlap
         the output DMAs with the second half of the main loop
"""

import os
import sys

import numpy as np

for _p in ("/opt/trn_rl_repo", os.path.expanduser("~/.axon_site/_ro/trn_rl_repo")):
    if os.path.isdir(_p) and _p not in sys.path:
        sys.path.insert(0, _p)
        break

import concourse.bass as bass
import concourse.tile as tile
from concourse import bacc, mybir
from concourse.bass_utils import run_bass_kernel_spmd

A, B, C, N = 1024, 128, 16, 256
NCORES = 8
BL = B // NCORES          # 16 b-channels per core
BC = BL * C               # 256 (b, c) pairs per core
NGROUPS = N // 8          # 32 groups of 8 k-values
F32 = mybir.dt.float32
ALU = mybir.AluOpType
AF = mybir.ActivationFunctionType

DT_STREAM = mybir.dt.float16  # dtype of the absdiff stream path (F32 or float16)
F32R = mybir.dt.float32r


def on_act(k: int) -> bool:
    """k's whose relu-diff runs on ScalarE (load balancing vs DVE)."""
    return k % 8 == 3 and k % 32 != 27


_cache = {}


def _patterns():
    # patA maps bc-block0 partitions (b = p//16 in 0..7) to out row b;
    # patB maps bc-block1 partitions to out rows 8 + p//16.
    patA = np.zeros((128, 16), np.float32)
    patB = np.zeros((128, 16), np.float32)
    for p in range(128):
        patA[p, p // 16] = 1.0
        patB[p, 8 + p // 16] = 1.0
    # fold weight: out[32g + b, :] += rhs[b, :]
    foldW = np.zeros((16, 128), np.float32)
    for m in range(128):
        if m % 32 < 16:
            foldW[m % 32, m] = 1.0
    return patA, patB, foldW


def build_program(dbg: bool = False):
    nc = bacc.Bacc(
        "TRN2", target_bir_lowering=False, debug=False, enable_asserts=True
    )

    xT_d = nc.dram_tensor("xT", [A, N], DT_STREAM, kind="ExternalInput")
    tl_d = nc.dram_tensor("Tl", [A, BC], DT_STREAM, kind="ExternalInput")
    out_d = nc.dram_tensor("out", [N, BL], F32, kind="ExternalOutput")
    if dbg:
        mt_o = nc.dram_tensor("mt_o", [2, 128, N], F32, kind="ExternalOutput")
        negR_o = nc.dram_tensor("negR_o", [16, N], F32, kind="ExternalOutput")
        psD_o = nc.dram_tensor("psD_o", [2, 128, 512], F32, kind="ExternalOutput")
        sall_o = nc.dram_tensor("sall_o", [128, 64], F32, kind="ExternalOutput")
        c255_o = nc.dram_tensor("c255_o", [128, 64], F32, kind="ExternalOutput")

    np_dt = np.float32 if DT_STREAM == F32 else np.float16
    patA_np, patB_np, foldW_np = _patterns()
    pats_np = np.concatenate(
        [patA_np, patB_np, 2 * patA_np, 2 * patB_np], axis=1)  # [128, 64]
    pats_d = nc.inline_tensor(pats_np.astype(np_dt), name="pats")
    foldW_d = nc.inline_tensor(foldW_np.astype(np_dt), name="foldW")

    xT_v = xT_d[:].rearrange("(a p) n -> p a n", p=128)
    tl_v = tl_d[:].rearrange("(a p) m -> p a m", p=128)

    with tile.TileContext(nc) as tc:
        with (
            tc.tile_pool(name="persist", bufs=1) as pp,
            tc.tile_pool(name="ad", bufs=32) as adp,
            tc.tile_pool(name="cm", bufs=8) as cmp_,
            tc.tile_pool(name="psum_d", bufs=6, space="PSUM") as pd,
        ):
            # ---- PE clock warmup: the HAM gate holds PE at half clock
            # until ~3.5us of sustained activity; PE would idle during the
            # input DMAs anyway, so burn that window on dummy matmuls and
            # run the real GEMM at full clock ----
            warm_t = pp.tile([128, 128], DT_STREAM, tag="warm")
            nc.vector.memset(warm_t[:], 0.0)
            pwm_ctx = tc.tile_pool(name="psum_warm", bufs=1, space="PSUM")
            pwm = pwm_ctx.__enter__()
            wps = pwm.tile([128, 128], F32, tag="wps")
            for _ in range(32):
                nc.tensor.matmul(wps[:], warm_t[:], warm_t[:],
                                 start=True, stop=True)
            pwm_ctx.__exit__(None, None, None)

            # ---- load inputs (split across both HWDGE rings: SP and ACT) ----
            xbig = pp.tile([128, 8 * N], DT_STREAM, tag="xbig")
            tbig = pp.tile([128, 8 * BC], DT_STREAM, tag="tbig")
            xbv = xbig[:].rearrange("p (a n) -> p a n", a=8)
            tbv = tbig[:].rearrange("p (a m) -> p a m", a=8)
            # first a-chunk alone so the GEMM can start ~1.5us in
            nc.sync.dma_start(xbv[:, 0:1], xT_v[:, 0:1])
            nc.scalar.dma_start(tbv[:, 0:1], tl_v[:, 0:1])
            nc.sync.dma_start(xbv[:, 1:4], xT_v[:, 1:4])
            nc.scalar.dma_start(tbv[:, 1:4], tl_v[:, 1:4])
            nc.scalar.dma_start(xbv[:, 4:8], xT_v[:, 4:8])
            nc.sync.dma_start(tbv[:, 4:8], tl_v[:, 4:8])
            xts = [xbig[:, a * N:(a + 1) * N] for a in range(8)]
            tls = [tbig[:, a * BC:(a + 1) * BC] for a in range(8)]

            pats_t = pp.tile([128, 64], DT_STREAM, tag="pats")
            nc.sync.dma_start(pats_t[:], pats_d[:])
            pats = {
                "patA1": pats_t[:, 0:16], "patB1": pats_t[:, 16:32],
                "patA2": pats_t[:, 32:48], "patB2": pats_t[:, 48:64],
            }
            foldW_t = pp.tile([16, 128], DT_STREAM, tag="foldW")
            nc.scalar.dma_start(foldW_t[:], foldW_d[:])

            # ---- GEMM: MT[bc, n] = sum_a Tl[a, bc] * x[n, a] ----
            pmt_ctx = tc.tile_pool(name="psum_mt", bufs=2, space="PSUM")
            pmt = pmt_ctx.__enter__()
            MT = []        # stream dtype (input of absdiff)
            MTs = []       # f32 scalar source for tensor_scalar scalar1
            negMT = []     # f32, bias source for ScalarE Abs
            for blk in range(2):
                ps = pmt.tile([128, N], F32, tag="psmt")
                for a in range(8):
                    nc.tensor.matmul(
                        ps[:],
                        tls[a][:, blk * 128:(blk + 1) * 128],
                        xts[a],
                        start=(a == 0),
                        stop=(a == 7),
                    )
                mt_t = pp.tile([128, N], DT_STREAM, tag=f"mt{blk}")
                nc.scalar.copy(mt_t[:], ps[:])
                if DT_STREAM == F32:
                    mts_t = mt_t
                else:
                    mts_t = pp.tile([128, N], F32, tag=f"mts{blk}")
                    nc.vector.tensor_copy(mts_t[:], mt_t[:])
                nmt_t = pp.tile([128, N], F32, tag=f"nmt{blk}")
                nc.vector.tensor_scalar(
                    out=nmt_t[:], in0=mts_t[:], scalar1=-1.0, scalar2=None,
                    op0=ALU.mult,
                )
                MT.append(mt_t)
                MTs.append(mts_t)
                negMT.append(nmt_t)

            # ---- R[b, j] = sum_c M[j, b, c]; negR = -R ----
            psR = pmt.tile([16, N], F32, tag="psmt")
            nc.tensor.matmul(psR[:], pats["patA1"], MT[0][:],
                             start=True, stop=False)
            nc.tensor.matmul(psR[:], pats["patB1"], MT[1][:],
                             start=False, stop=True)
            # negR in stream dtype: the fold matmul adds exactly these values,
            # and the exp bias below must cancel them bit-exactly on j == k.
            negR = pp.tile([16, N], DT_STREAM, tag="negR")
            nc.scalar.mul(negR[:], psR[:], -1.0)
            pmt_ctx.__exit__(None, None, None)

            # negRbias: [128, 64]; col 2G+h rows 32g+b = -R[b, 8G+4h+g]
            negRb = pp.tile([128, 2 * NGROUPS], DT_STREAM, tag="negRb")
            nc.vector.memset(negRb[:], 0.0)
            for g in range(4):
                src = negR[:].rearrange("b (q g) -> b q g", g=4)[:, :, g]
                nc.sync.dma_start(negRb[32 * g:32 * g + 16, :], src)

            # ---- persistent result tiles ----
            S_all = pp.tile([128, 2 * NGROUPS], F32, tag="S_all")
            C255 = pp.tile([128, 2 * NGROUPS], F32, tag="C255")
            C00 = pp.tile([16, 1], F32, tag="C00")
            R_all = pp.tile([128, 2 * NGROUPS], F32, tag="R_all")
            # out row k = 8G+4h+g, col b  <-  R_all[32g+b, 2G+h]
            dstv = out_d[:].rearrange("(G h g) b -> g b G h", G=NGROUPS, h=2, g=4)

            def make_act_ads(GG, store):
                for h in range(2):
                    for g in range(4):
                        k = 8 * GG + 4 * h + g
                        if not on_act(k):
                            continue
                        pair = []
                        for blk in range(2):
                            ad_t = adp.tile([128, N], DT_STREAM, tag="adact")
                            nc.scalar.activation(
                                ad_t[:], MT[blk][:], AF.Relu,
                                bias=negMT[blk][:, k:k + 1], scale=1.0,
                            )
                            pair.append(ad_t)
                        store[k] = pair

            def finalize_half(lo, hi):
                # out[k] = S[k] - Cmat[k, 255] for cols [lo, hi)
                nc.vector.tensor_tensor(
                    out=R_all[:, lo:hi], in0=S_all[:, lo:hi],
                    in1=C255[:, lo:hi], op=ALU.subtract,
                )
                if lo == 0:
                    # out[0] = S[0] - Cmat[0, 0]
                    nc.vector.tensor_tensor(
                        out=R_all[0:16, 0:1], in0=S_all[0:16, 0:1],
                        in1=C00[:], op=ALU.subtract,
                    )
                # (k=255 -> col 63: its C255 value IS Cmat[255,255]; no fix)
                for g in range(4):
                    srcv = R_all[32 * g:32 * g + 16, lo:hi].rearrange(
                        "b (G h) -> b G h", h=2)
                    nc.sync.dma_start(dstv[g][:, lo // 2:hi // 2, :], srcv)

            # ---- main loop ----
            act_ads = {}
            make_act_ads(0, act_ads)
            for G in range(NGROUPS):
                if G + 1 < NGROUPS:
                    # ScalarE relu-diffs for the NEXT group, queued ahead of
                    # this group's exps so a blocked exp can't delay them
                    make_act_ads(G + 1, act_ads)
                if G == NGROUPS // 2:
                    finalize_half(0, NGROUPS)
                for h in range(2):
                    # own PSUM bank per half: the exp (ACT read) must not
                    # share a bank with the next half's PE writes, or Tile
                    # serializes them (bank-overlap tracking).
                    psDh = pd.tile([128, N], F32, tag="psD")
                    for g in range(4):
                        k = 8 * G + 4 * h + g
                        if on_act(k):
                            ads = act_ads.pop(k)
                        else:
                            ads = []
                            for blk in range(2):
                                ad_t = adp.tile([128, N], DT_STREAM, tag="ad")
                                nc.vector.tensor_scalar(
                                    out=ad_t[:], in0=MT[blk][:],
                                    scalar1=MTs[blk][:, k:k + 1], scalar2=0.0,
                                    op0=ALU.subtract, op1=ALU.max,
                                )
                                ads.append(ad_t)
                        outsl = psDh[32 * g:32 * g + 16, :]
                        nc.tensor.matmul(
                            outsl, pats["patA2"], ads[0][:],
                            start=True, stop=False, tile_position=(0, 32 * g),
                        )
                        nc.tensor.matmul(
                            outsl, pats["patB2"], ads[1][:],
                            start=False, stop=False,
                            tile_position=(0, 32 * g),
                        )
                    # psDh += -R[b, j] broadcast over the four 32-row groups
                    nc.tensor.matmul(
                        psDh[:], foldW_t[:], negR[:],
                        start=False, stop=True, skip_group_check=True,
                    )
                    if dbg and G in (0, 4):
                        dcp = cmp_.tile([128, N], F32, tag="dcp")
                        nc.vector.tensor_copy(dcp[:], psDh[:])
                        nc.sync.dma_start(
                            psD_o[:][0 if G == 0 else 1][:, N * h:N * (h + 1)],
                            dcp[:])
                    col = 2 * G + h
                    cm_t = cmp_.tile([128, N], F32, tag="cm")
                    nc.scalar.activation(
                        cm_t[:], psDh[:], AF.Exp,
                        bias=negRb[:, col:col + 1],
                        scale=-1.0,
                        accum_out=S_all[:, col:col + 1],
                    )
                    nc.gpsimd.tensor_copy(C255[:, col:col + 1], cm_t[:, 255:256])
                    if G == 0 and h == 0:
                        # Cmat[0, 0, :] lives at rows 0..15, j-col 0 (k=0 is g=0)
                        nc.gpsimd.tensor_copy(C00[:], cm_t[0:16, 0:1])

            finalize_half(NGROUPS, 2 * NGROUPS)

            if dbg:
                for blk in range(2):
                    nc.sync.dma_start(mt_o[:][blk], MTs[blk][:])
                nc.sync.dma_start(negR_o[:], negR[:])
                nc.sync.dma_start(sall_o[:], S_all[:])
                nc.sync.dma_start(c255_o[:], C255[:])

    nc.compile()
    return nc


def kernel(x: np.ndarray, T: np.ndarray) -> np.ndarray:
    if "nc" not in _cache:
        _cache["nc"] = build_program()
    nc = _cache["nc"]

    np_dt = np.float32 if DT_STREAM == F32 else np.float16
    x = np.ascontiguousarray(x, dtype=np.float32)
    T = np.ascontiguousarray(T, dtype=np.float32)
    xT = np.ascontiguousarray(x.T.astype(np_dt))         # [A, N]

    in_maps = []
    for c in range(NCORES):
        tl = np.ascontiguousarray(
            T[:, c * BL:(c + 1) * BL, :].reshape(A, BC).astype(np_dt))
        in_maps.append({"xT": xT, "Tl": tl})

    res = run_bass_kernel_spmd(nc, in_maps, list(range(NCORES)))
    outs = [res.results[c]["out"] for c in range(NCORES)]
    return np.concatenate(outs, axis=1)                  # [N, B]


if __name__ == "__main__":
    rng = np.random.default_rng(0)
    x = rng.standard_normal((N, A)).astype(np.float32)
    T = rng.random((A, B, C), dtype=np.float32)
    out = kernel(x, T)
    print(out.shape, out.dtype, out[:3, :3])



# revision 13
# speedup vs baseline: 1.2088x; 1.2088x over previous
"""MiniBatchDiscrimination kernel for 8 Trainium2 NeuronCores.

Problem:
  x [256, 1024] f32, T [1024, 128, 16] f32
  M = einsum('na,abc->nbc', x, T)                      [N=256, B=128, C=16]
  D[k,j,b] = sum_c |M[k,b,c] - M[j,b,c]|
  Cmat = exp(-D); S = sum_j Cmat
  out = S - Cmat[:, N-1, :]; out[0] = S[0]-Cmat[0,0]; out[N-1] = S[N-1]-Cmat[N-1,N-1]

Sharding: data-parallel over B (each core owns 16 of the 128 b-channels).
The pairwise distance is independent per b, so there is no communication.

Per-core dataflow ("transposed-D" design). |d| = 2*relu(d) - d with the
linear term folded in via R[b,k] = sum_c M[k,b,c]:
  D[k,j,b] = 2*P[k,j,b] - R[b,j] + R[b,k],  P = sum_c relu(M[j]-M[k])

  PE   : MT[bc, n] = (x @ T_loc)^T (16 accumulating matmuls)
  3-way: relu-diff tiles ad_k[bc, j] = relu(MT[:, j] - MT[:, k]) split
         across DVE tensor_scalar (4x mode), ScalarE activation(Relu),
         and Pool tensor_scalar -- the elementwise work is the wall.
  PE   : c-reduction with the relu tile as the STATIONARY operand and the
         tiny one-hot pattern as the moving tensor: out[j, (b,kk)] slices
         of a [128 j, 512 (b,kk)] PSUM bank cost only 8 columns each.
         +R[b,k] / -R[b,j] are folded by two more cheap matmuls, so the
         self column is exactly 0 and exp needs no bias.
  ScE  : Cmat tile = exp(-psD) over [128, 512] (no bias, no accum).
  PE   : row sums S[k,b] = sum_j Cmat via one-hot-column stationaries into
         one [16, 512] PSUM bank; the jh=1 stationary has row 127 zeroed,
         which folds the "exclude j=255" output quirk for every k.
  fix  : out[0] needs + (Cmat[0,255]-Cmat[0,0]); two [1,16] row extracts
         (Pool) + one DVE add into the S bank.
  out  : S -> SBUF copy with k-major free-dim permutation -> one 8-descriptor
         DMA to DRAM.
"""

import os
import sys

import numpy as np

for _p in ("/opt/trn_rl_repo", os.path.expanduser("~/.axon_site/_ro/trn_rl_repo")):
    if os.path.isdir(_p) and _p not in sys.path:
        sys.path.insert(0, _p)
        break

import concourse.bass as bass
import concourse.tile as tile
from concourse import bacc, mybir
from concourse.bass_utils import run_bass_kernel_spmd

A, B, C, N = 1024, 128, 16, 256
NCORES = 8
BL = B // NCORES          # 16 b-channels per core
BC = BL * C               # 256 (b, c) pairs per core
NG = 8                    # k-groups
GK = N // NG              # 32 k's per group
F32 = mybir.dt.float32
F16 = mybir.dt.float16
ALU = mybir.AluOpType
AF = mybir.ActivationFunctionType

# per-group engine split for the 32 (k -> 2 relu-diff ops) slots
ACT_KK = {2, 8, 15, 21, 27}
POOL_KK = {5, 11, 17, 23, 29, 31}

_cache = {}


def _consts():
    # patM: moving tensor of the c-reduction; patM[p, j] = 2*(p//16 == j)
    patM = np.zeros((128, 8), np.float32)
    for p in range(128):
        patM[p, p // 16] = 2.0
    # patA1/patB1: R row-sum patterns (psR[b, j] = sum_c M[j, 16b+c])
    patA1 = np.zeros((128, 16), np.float32)
    patB1 = np.zeros((128, 16), np.float32)
    for p in range(128):
        patA1[p, p // 16] = 1.0
        patB1[p, 8 + p // 16] = 1.0
    # selT: S-accumulation stationaries; col 16*(2g+jh)+m = (m==g), with
    # row 127 zeroed for jh=1 (drops j=255 from every row sum). Row m=8 of
    # the g=0 stationaries is the special k=0 sum: drop j=0, keep j=255.
    selT = np.zeros((128, 16 * 2 * NG), np.float32)
    for g in range(NG):
        for jh in range(2):
            col = 16 * (2 * g + jh) + g
            selT[:, col] = 1.0
            if jh == 1:
                selT[127, col] = 0.0
    selT[1:, 16 * 0 + 8] = 1.0    # g=0, jh=0: j in [1, 128)
    selT[:, 16 * 1 + 8] = 1.0     # g=0, jh=1: j in [128, 256] all
    # onehotB: -R[b,j] fold moving tensor; onehotB[b', 32b+kk] = (b==b')
    onehotB = np.zeros((16, 512), np.float32)
    for b in range(16):
        onehotB[b, 32 * b:32 * b + 32] = 1.0
    return patM, patA1, patB1, selT, onehotB


def build_program(dbg: bool = False, dbg_parts: int = 7):
    # dbg_parts bitmask: 1=pattern matmuls, 2=+R_k fold, 4=-R_j fold
    nc = bacc.Bacc(
        "TRN2", target_bir_lowering=False, debug=False, enable_asserts=True
    )

    xT_d = nc.dram_tensor("xT", [A, N], F16, kind="ExternalInput")
    tl_d = nc.dram_tensor("Tl", [A, BC], F16, kind="ExternalInput")
    out_d = nc.dram_tensor("out", [N, BL], F32, kind="ExternalOutput")
    if dbg:
        mt_o = nc.dram_tensor("mt_o", [2, 128, N], F32, kind="ExternalOutput")
        prf_o = nc.dram_tensor("prf_o", [1, 16 * N], F32, kind="ExternalOutput")
        psd_o = nc.dram_tensor("psd_o", [2, 128, 512], F32,
                               kind="ExternalOutput")
        cm_o = nc.dram_tensor("cm_o", [2, 128, 512], F32,
                              kind="ExternalOutput")
        s_o = nc.dram_tensor("s_o", [16, 512], F32, kind="ExternalOutput")

    patM_np, patA1_np, patB1_np, selT_np, onehotB_np = _consts()
    cA_np = np.concatenate([patM_np, patA1_np, patB1_np, selT_np], axis=1)
    cA_d = nc.inline_tensor(cA_np.astype(np.float16), name="cA")
    cB_d = nc.inline_tensor(onehotB_np.astype(np.float16), name="cB")

    xT_v = xT_d[:].rearrange("(a p) n -> p a n", p=128)
    tl_v = tl_d[:].rearrange("(a p) m -> p a m", p=128)

    with tile.TileContext(nc) as tc:
        with (
            tc.tile_pool(name="persist", bufs=1) as pp,
            tc.tile_pool(name="ad", bufs=28) as adp,
            tc.tile_pool(name="cm", bufs=4) as cmp_,
            tc.tile_pool(name="psum_d", bufs=5, space="PSUM") as pd,
            tc.tile_pool(name="psum_s", bufs=1, space="PSUM") as psS,
        ):
            # ---- PE clock warmup (HAM gate holds PE at low clock for
            # ~3.5us; burn the input-DMA window on dummy matmuls) ----
            warm_t = pp.tile([128, 128], F16, tag="warm")
            nc.vector.memset(warm_t[:], 0.0)
            ones1 = pp.tile([1, 128], F16, tag="ones1")
            nc.vector.memset(ones1[:], 1.0)
            pwm_ctx = tc.tile_pool(name="psum_warm", bufs=1, space="PSUM")
            pwm = pwm_ctx.__enter__()
            wps = pwm.tile([128, 128], F32, tag="wps")
            for _ in range(32):
                nc.tensor.matmul(wps[:], warm_t[:], warm_t[:],
                                 start=True, stop=True)
            pwm_ctx.__exit__(None, None, None)

            # ---- load inputs (split across the SP and ACT HWDGE rings) ----
            xbig = pp.tile([128, 8 * N], F16, tag="xbig")
            tbig = pp.tile([128, 8 * BC], F16, tag="tbig")
            xbv = xbig[:].rearrange("p (a n) -> p a n", a=8)
            tbv = tbig[:].rearrange("p (a m) -> p a m", a=8)
            nc.sync.dma_start(xbv[:, 0:1], xT_v[:, 0:1])
            nc.scalar.dma_start(tbv[:, 0:1], tl_v[:, 0:1])
            nc.sync.dma_start(xbv[:, 1:4], xT_v[:, 1:4])
            nc.scalar.dma_start(tbv[:, 1:4], tl_v[:, 1:4])
            nc.scalar.dma_start(xbv[:, 4:8], xT_v[:, 4:8])
            nc.sync.dma_start(tbv[:, 4:8], tl_v[:, 4:8])
            xts = [xbig[:, a * N:(a + 1) * N] for a in range(8)]
            tls = [tbig[:, a * BC:(a + 1) * BC] for a in range(8)]

            cA_t = pp.tile([128, 8 + 16 + 16 + 32 * NG], F16, tag="cA")
            nc.sync.dma_start(cA_t[:], cA_d[:])
            patM = cA_t[:, 0:8]
            patA1 = cA_t[:, 8:24]
            patB1 = cA_t[:, 24:40]
            selT = cA_t[:, 40:40 + 32 * NG]
            onehotB = pp.tile([16, 512], F16, tag="cB")
            nc.scalar.dma_start(onehotB[:], cB_d[:])

            # ---- GEMM: MT[bc, n] = sum_a Tl[a, bc] * x[n, a] ----
            pmt_ctx = tc.tile_pool(name="psum_mt", bufs=2, space="PSUM")
            pmt = pmt_ctx.__enter__()
            MT = []        # f16 stream (relu-diff input, PE stationaries)
            MTs = []       # f32 scalar source for tensor_scalar scalar1
            negMT = []     # f32 bias source for ScalarE Relu
            for blk in range(2):
                ps = pmt.tile([128, N], F32, tag="psmt")
                for a in range(8):
                    nc.tensor.matmul(
                        ps[:],
                        tls[a][:, blk * 128:(blk + 1) * 128],
                        xts[a],
                        start=(a == 0),
                        stop=(a == 7),
                    )
                mt_t = pp.tile([128, N], F16, tag=f"mt{blk}")
                nc.scalar.copy(mt_t[:], ps[:])
                mts_t = pp.tile([128, N], F32, tag=f"mts{blk}")
                nc.vector.tensor_copy(mts_t[:], mt_t[:])
                nmt_t = pp.tile([128, N], F32, tag=f"nmt{blk}")
                nc.vector.tensor_scalar(
                    out=nmt_t[:], in0=mts_t[:], scalar1=-1.0, scalar2=None,
                    op0=ALU.mult,
                )
                MT.append(mt_t)
                MTs.append(mts_t)
                negMT.append(nmt_t)

            # ---- R[b, j] = sum_c M[j, b, c] ----
            psR = pmt.tile([16, N], F32, tag="psmt")
            nc.tensor.matmul(psR[:], patA1, MT[0][:], start=True, stop=False)
            nc.tensor.matmul(psR[:], patB1, MT[1][:], start=False, stop=True)
            # posR/negR in f16: the +R_k and -R_j folds add exactly these
            # values, so the j==k column cancels bit-exactly.
            posR = pp.tile([16, N], F16, tag="posR")
            nc.scalar.copy(posR[:], psR[:])
            negR = pp.tile([16, N], F16, tag="negR")
            nc.scalar.mul(negR[:], psR[:], -1.0)
            pmt_ctx.__exit__(None, None, None)

            # posRflat[0, 256*b + k] = R[b, k]  (moving tensor of +R_k fold
            # must start at partition 0; one 16-descriptor SBUF->SBUF DMA)
            posRflat = pp.tile([1, 16 * N], F16, tag="posRflat")
            dstf = posRflat[:].rearrange("p (b k) -> p b k", b=16)
            nc.sync.dma_start(dstf, posR[:])

            # ---- persistent result tiles ----
            S_ps = psS.tile([16, 512], F32, tag="S")
            Ssb = pp.tile([16, 512], F32, tag="Ssb")

            def emit_ads(G):
                """relu-diff tiles for group G, split across engines."""
                ads = {}
                for kk in range(GK):
                    k = GK * G + kk
                    pair = []
                    for blk in range(2):
                        ad_t = adp.tile([128, N], F16, tag="ad")
                        if kk in ACT_KK:
                            nc.scalar.activation(
                                ad_t[:], MT[blk][:], AF.Relu,
                                bias=negMT[blk][:, k:k + 1], scale=1.0,
                            )
                        elif kk in POOL_KK:
                            nc.gpsimd.tensor_scalar(
                                out=ad_t[:], in0=MT[blk][:],
                                scalar1=MTs[blk][:, k:k + 1], scalar2=0.0,
                                op0=ALU.subtract, op1=ALU.max,
                            )
                        else:
                            nc.vector.tensor_scalar(
                                out=ad_t[:], in0=MT[blk][:],
                                scalar1=MTs[blk][:, k:k + 1], scalar2=0.0,
                                op0=ALU.subtract, op1=ALU.max,
                            )
                        pair.append(ad_t)
                    ads[kk] = pair
                return ads

            def emit_banks(G, ads):
                """c-reduction + R folds for group G -> two [128,512] banks."""
                banks = []
                for jh in range(2):
                    psD = pd.tile([128, 512], F32, tag="psD")
                    psDv = psD[:].rearrange("p (b k) -> p k b", b=16)
                    # start=True zeroes the whole bank row of every written
                    # partition, so exactly ONE start=True covering all 512
                    # cols: the -R[b,j] fold goes first.
                    nc.tensor.matmul(
                        psD[:], negR[:, 128 * jh:128 * (jh + 1)], onehotB[:],
                        start=True, stop=False,
                    )
                    for b in range(16):
                        nc.tensor.matmul(
                            psD[:, 32 * b:32 * b + 32],
                            ones1[:],
                            posRflat[:, 256 * b + GK * G:
                                      256 * b + GK * G + GK],
                            start=False, stop=False, skip_group_check=True,
                        )
                    for kk in range(GK):
                        for blk in range(2):
                            outsl = psDv[:, kk, 8 * blk:8 * blk + 8]
                            nc.tensor.matmul(
                                outsl,
                                ads[kk][blk][:, 128 * jh:128 * (jh + 1)],
                                patM,
                                start=False,
                                stop=(kk == GK - 1 and blk == 1),
                                skip_group_check=True,
                            )
                    banks.append(psD)
                return banks

            def emit_tail(G, banks):
                """exp + S row-sum for group G."""
                for jh in range(2):
                    cm_t = cmp_.tile([128, 512], F16, tag="cm")
                    if dbg and G == 0:
                        dcp = cmp_.tile([128, 512], F32, tag="dcp")
                        nc.vector.tensor_copy(dcp[:], banks[jh][:])
                        nc.sync.dma_start(psd_o[:][jh], dcp[:])
                    nc.scalar.activation(cm_t[:], banks[jh][:], AF.Exp,
                                         scale=-1.0)
                    if dbg and G == 0:
                        dcm = cmp_.tile([128, 512], F32, tag="dcm")
                        nc.vector.tensor_copy(dcm[:], cm_t[:])
                        nc.sync.dma_start(cm_o[:][jh], dcm[:])
                    nc.tensor.matmul(
                        S_ps[:], selT[:, 32 * G + 16 * jh:32 * G + 16 * jh + 16],
                        cm_t[:],
                        start=(G == 0 and jh == 0),
                        stop=(G == NG - 1 and jh == 1),
                        skip_group_check=not (G == 0 and jh == 0),
                    )

            # ---- main loop (exp/S of group G-1 emitted after ads of G so a
            # blocked exp can't stall the ScalarE relu stream) ----
            prev = None
            for G in range(NG):
                ads = emit_ads(G)
                banks = emit_banks(G, ads)
                if prev is not None:
                    emit_tail(G - 1, prev)
                prev = banks
            emit_tail(NG - 1, prev)

            # ---- output: copy with (b,kk)->(kk,b) free permutation so the
            # DRAM DMAs are a few contiguous descriptors. Row 8 holds the
            # special k=0 sum; out row 0 comes from it via the third DMA. ----
            nc.scalar.copy(Ssb[:],
                           S_ps[:].rearrange("g (b k) -> g k b", b=16))
            dstv = out_d[:].rearrange("(g k) b -> g (k b)", g=8)
            nc.sync.dma_start(dstv[0:1, 16:512], Ssb[0:1, 16:512])
            nc.sync.dma_start(dstv[1:8], Ssb[1:8, :])
            nc.sync.dma_start(out_d[0:1, :], Ssb[8:9, 0:16])

            if dbg:
                for blk in range(2):
                    nc.sync.dma_start(mt_o[:][blk], MTs[blk][:])
                prf32 = pp.tile([1, 16 * N], F32, tag="prf32")
                nc.vector.tensor_copy(prf32[:], posRflat[:])
                nc.sync.dma_start(prf_o[:], prf32[:])
                nc.sync.dma_start(s_o[:], Ssb[:])

    nc.compile()
    return nc


def kernel(x: np.ndarray, T: np.ndarray) -> np.ndarray:
    if "nc" not in _cache:
        _cache["nc"] = build_program()
    nc = _cache["nc"]

    x = np.ascontiguousarray(x, dtype=np.float32)
    T = np.ascontiguousarray(T, dtype=np.float32)
    xT = np.ascontiguousarray(x.T.astype(np.float16))  # [A, N]

    in_maps = []
    for c in range(NCORES):
        tl = np.ascontiguousarray(
            T[:, c * BL:(c + 1) * BL, :].reshape(A, BC).astype(np.float16))
        in_maps.append({"xT": xT, "Tl": tl})

    res = run_bass_kernel_spmd(nc, in_maps, list(range(NCORES)))
    outs = [res.results[c]["out"] for c in range(NCORES)]
    return np.concatenate(outs, axis=1)                  # [N, B]


if __name__ == "__main__":
    rng = np.random.default_rng(0)
    x = rng.standard_normal((N, A)).astype(np.float32)
    T = rng.random((A, B, C), dtype=np.float32)
    out = kernel(x, T)
    print(out.shape, out.dtype, out[:3, :3])


# revision 16
# speedup vs baseline: 1.2533x; 1.0369x over previous
"""MiniBatchDiscrimination kernel for 8 Trainium2 NeuronCores.

Problem:
  x [256, 1024] f32, T [1024, 128, 16] f32
  M = einsum('na,abc->nbc', x, T)                      [N=256, B=128, C=16]
  D[k,j,b] = sum_c |M[k,b,c] - M[j,b,c]|
  Cmat = exp(-D); S = sum_j Cmat
  out = S - Cmat[:, N-1, :]; out[0] = S[0]-Cmat[0,0]; out[N-1] = S[N-1]-Cmat[N-1,N-1]

Sharding: data-parallel over B (each core owns 16 of the 128 b-channels).
The pairwise distance is independent per b, so there is no communication.

Per-core dataflow ("transposed-D" design). |d| = 2*relu(d) - d with the
linear term folded in via R[b,k] = sum_c M[k,b,c]:
  D[k,j,b] = 2*P[k,j,b] - R[b,j] + R[b,k],  P = sum_c relu(M[j]-M[k])

  PE   : MT[bc, n] = (x @ T_loc)^T (16 accumulating matmuls)
  3-way: relu-diff tiles ad_k[bc, j] = relu(MT[:, j] - MT[:, k]) split
         across DVE tensor_scalar (4x mode), ScalarE activation(Relu),
         and Pool tensor_scalar -- the elementwise work is the wall.
  PE   : c-reduction with the relu tile as the STATIONARY operand and the
         tiny one-hot pattern as the moving tensor: out[j, (b,kk)] slices
         of a [128 j, 512 (b,kk)] PSUM bank cost only 8 columns each.
         +R[b,k] / -R[b,j] are folded by two more cheap matmuls, so the
         self column is exactly 0 and exp needs no bias.
  ScE  : Cmat tile = exp(-psD) over [128, 512] (no bias, no accum).
  PE   : row sums S[k,b] = sum_j Cmat via one-hot-column stationaries into
         one [16, 512] PSUM bank; the jh=1 stationary has row 127 zeroed,
         which folds the "exclude j=255" output quirk for every k.
  fix  : out[0] needs + (Cmat[0,255]-Cmat[0,0]); two [1,16] row extracts
         (Pool) + one DVE add into the S bank.
  out  : S -> SBUF copy with k-major free-dim permutation -> one 8-descriptor
         DMA to DRAM.
"""

import os
import sys

import numpy as np

for _p in ("/opt/trn_rl_repo", os.path.expanduser("~/.axon_site/_ro/trn_rl_repo")):
    if os.path.isdir(_p) and _p not in sys.path:
        sys.path.insert(0, _p)
        break

import concourse.bass as bass
import concourse.tile as tile
from concourse import bacc, mybir
from concourse.bass_utils import run_bass_kernel_spmd

A, B, C, N = 1024, 128, 16, 256
NCORES = 8
BL = B // NCORES          # 16 b-channels per core
BC = BL * C               # 256 (b, c) pairs per core
NG = 8                    # k-groups
GK = N // NG              # 32 k's per group
F32 = mybir.dt.float32
F16 = mybir.dt.float16
ALU = mybir.AluOpType
AF = mybir.ActivationFunctionType

# per-group engine split for the 32 (k -> 2 relu-diff ops) slots
ACT_KK = {2, 8, 15, 21, 27}
POOL_KK = {5, 11, 17, 23, 29, 31}

_cache = {}


def _consts():
    # patM: moving tensor of the c-reduction; patM[p, j] = 2*(p//16 == j)
    patM = np.zeros((128, 8), np.float32)
    for p in range(128):
        patM[p, p // 16] = 2.0
    # patA1/patB1: R row-sum patterns (psR[b, j] = sum_c M[j, 16b+c])
    patA1 = np.zeros((128, 16), np.float32)
    patB1 = np.zeros((128, 16), np.float32)
    for p in range(128):
        patA1[p, p // 16] = 1.0
        patB1[p, 8 + p // 16] = 1.0
    # selT: S-accumulation stationaries; col 16*(2g+jh)+m = (m==g), with
    # row 127 zeroed for jh=1 (drops j=255 from every row sum). Row m=8 of
    # the g=0 stationaries is the special k=0 sum: drop j=0, keep j=255.
    selT = np.zeros((128, 16 * 2 * NG), np.float32)
    for g in range(NG):
        for jh in range(2):
            col = 16 * (2 * g + jh) + g
            selT[:, col] = 1.0
            if jh == 1:
                selT[127, col] = 0.0
    selT[1:, 16 * 0 + 8] = 1.0    # g=0, jh=0: j in [1, 128)
    selT[:, 16 * 1 + 8] = 1.0     # g=0, jh=1: j in [128, 256] all
    # onehotB: -R[b,j] fold moving tensor; onehotB[b', 32b+kk] = (b==b')
    onehotB = np.zeros((16, 512), np.float32)
    for b in range(16):
        onehotB[b, 32 * b:32 * b + 32] = 1.0
    return patM, patA1, patB1, selT, onehotB


def build_program(dbg: bool = False, dbg_parts: int = 7):
    # dbg_parts bitmask: 1=pattern matmuls, 2=+R_k fold, 4=-R_j fold
    nc = bacc.Bacc(
        "TRN2", target_bir_lowering=False, debug=False, enable_asserts=True
    )

    xT_d = nc.dram_tensor("xT", [A, N], F16, kind="ExternalInput")
    tl_d = nc.dram_tensor("Tl", [A, BC], F16, kind="ExternalInput")
    out_d = nc.dram_tensor("out", [N, BL], F32, kind="ExternalOutput")
    if dbg:
        mt_o = nc.dram_tensor("mt_o", [2, 128, N], F32, kind="ExternalOutput")
        prf_o = nc.dram_tensor("prf_o", [1, 16 * N], F32, kind="ExternalOutput")
        psd_o = nc.dram_tensor("psd_o", [2, 128, 512], F32,
                               kind="ExternalOutput")
        cm_o = nc.dram_tensor("cm_o", [2, 128, 512], F32,
                              kind="ExternalOutput")
        s_o = nc.dram_tensor("s_o", [16, 512], F32, kind="ExternalOutput")

    patM_np, patA1_np, patB1_np, selT_np, onehotB_np = _consts()
    cA_np = np.concatenate([patM_np, patA1_np, patB1_np, selT_np], axis=1)
    cA_d = nc.inline_tensor(cA_np.astype(np.float16), name="cA")
    cB_d = nc.inline_tensor(onehotB_np.astype(np.float16), name="cB")

    xT_v = xT_d[:].rearrange("(a p) n -> p a n", p=128)
    tl_v = tl_d[:].rearrange("(a p) m -> p a m", p=128)

    with tile.TileContext(nc) as tc:
        with (
            tc.tile_pool(name="persist", bufs=1) as pp,
            tc.tile_pool(name="ad", bufs=56) as adp,
            tc.tile_pool(name="cm", bufs=4) as cmp_,
            tc.tile_pool(name="psum_d", bufs=5, space="PSUM") as pd,
            tc.tile_pool(name="psum_s", bufs=1, space="PSUM") as psS,
        ):
            # ---- PE clock warmup (HAM gate holds PE at low clock for
            # ~3.5us; burn the input-DMA window on dummy matmuls) ----
            warm_t = pp.tile([128, 128], F16, tag="warm")
            nc.vector.memset(warm_t[:], 0.0)
            ones1 = pp.tile([1, 128], F16, tag="ones1")
            nc.vector.memset(ones1[:], 1.0)
            pwm_ctx = tc.tile_pool(name="psum_warm", bufs=1, space="PSUM")
            pwm = pwm_ctx.__enter__()
            wps = pwm.tile([128, 128], F32, tag="wps")
            for _ in range(32):
                nc.tensor.matmul(wps[:], warm_t[:], warm_t[:],
                                 start=True, stop=True)
            pwm_ctx.__exit__(None, None, None)

            # ---- load inputs (split across the SP and ACT HWDGE rings) ----
            xbig = pp.tile([128, 8 * N], F16, tag="xbig")
            tbig = pp.tile([128, 8 * BC], F16, tag="tbig")
            xbv = xbig[:].rearrange("p (a n) -> p a n", a=8)
            tbv = tbig[:].rearrange("p (a m) -> p a m", a=8)
            nc.sync.dma_start(xbv[:, 0:1], xT_v[:, 0:1])
            nc.scalar.dma_start(tbv[:, 0:1], tl_v[:, 0:1])
            nc.sync.dma_start(xbv[:, 1:4], xT_v[:, 1:4])
            nc.scalar.dma_start(tbv[:, 1:4], tl_v[:, 1:4])
            nc.scalar.dma_start(xbv[:, 4:8], xT_v[:, 4:8])
            nc.sync.dma_start(tbv[:, 4:8], tl_v[:, 4:8])
            xts = [xbig[:, a * N:(a + 1) * N] for a in range(8)]
            tls = [tbig[:, a * BC:(a + 1) * BC] for a in range(8)]

            cA_t = pp.tile([128, 8 + 16 + 16 + 32 * NG], F16, tag="cA")
            nc.sync.dma_start(cA_t[:], cA_d[:])
            patM = cA_t[:, 0:8]
            patA1 = cA_t[:, 8:24]
            patB1 = cA_t[:, 24:40]
            selT = cA_t[:, 40:40 + 32 * NG]
            onehotB = pp.tile([16, 512], F16, tag="cB")
            nc.scalar.dma_start(onehotB[:], cB_d[:])

            # ---- GEMM: MT[bc, n] = sum_a Tl[a, bc] * x[n, a] ----
            pmt_ctx = tc.tile_pool(name="psum_mt", bufs=2, space="PSUM")
            pmt = pmt_ctx.__enter__()
            MT = []        # f16 stream (relu-diff input, PE stationaries)
            MTs = []       # f32 scalar source for tensor_scalar scalar1
            negMT = []     # f32 bias source for ScalarE Relu
            for blk in range(2):
                ps = pmt.tile([128, N], F32, tag="psmt")
                for a in range(8):
                    nc.tensor.matmul(
                        ps[:],
                        tls[a][:, blk * 128:(blk + 1) * 128],
                        xts[a],
                        start=(a == 0),
                        stop=(a == 7),
                    )
                mt_t = pp.tile([128, N], F16, tag=f"mt{blk}")
                nc.scalar.copy(mt_t[:], ps[:])
                mts_t = pp.tile([128, N], F32, tag=f"mts{blk}")
                nc.vector.tensor_copy(mts_t[:], mt_t[:])
                nmt_t = pp.tile([128, N], F32, tag=f"nmt{blk}")
                nc.vector.tensor_scalar(
                    out=nmt_t[:], in0=mts_t[:], scalar1=-1.0, scalar2=None,
                    op0=ALU.mult,
                )
                MT.append(mt_t)
                MTs.append(mts_t)
                negMT.append(nmt_t)

            # ---- R[b, j] = sum_c M[j, b, c] ----
            psR = pmt.tile([16, N], F32, tag="psmt")
            nc.tensor.matmul(psR[:], patA1, MT[0][:], start=True, stop=False)
            nc.tensor.matmul(psR[:], patB1, MT[1][:], start=False, stop=True)
            # posR/negR in f16: the +R_k and -R_j folds add exactly these
            # values, so the j==k column cancels bit-exactly.
            posR = pp.tile([16, N], F16, tag="posR")
            nc.scalar.copy(posR[:], psR[:])
            negR = pp.tile([16, N], F16, tag="negR")
            nc.scalar.mul(negR[:], psR[:], -1.0)
            pmt_ctx.__exit__(None, None, None)

            # posRflat[0, 256*b + k] = R[b, k]  (moving tensor of +R_k fold
            # must start at partition 0; one 16-descriptor SBUF->SBUF DMA)
            posRflat = pp.tile([1, 16 * N], F16, tag="posRflat")
            dstf = posRflat[:].rearrange("p (b k) -> p b k", b=16)
            nc.sync.dma_start(dstf, posR[:])

            # ---- persistent result tiles ----
            S_ps = psS.tile([16, 512], F32, tag="S")
            Ssb = pp.tile([16, 512], F32, tag="Ssb")

            def emit_ads(G):
                """relu-diff tiles for group G, split across engines."""
                ads = {}
                for kk in range(GK):
                    k = GK * G + kk
                    pair = []
                    for blk in range(2):
                        ad_t = adp.tile([128, N], F16, tag="ad")
                        if kk in ACT_KK:
                            nc.scalar.activation(
                                ad_t[:], MT[blk][:], AF.Relu,
                                bias=negMT[blk][:, k:k + 1], scale=1.0,
                            )
                        elif kk in POOL_KK:
                            nc.gpsimd.tensor_scalar(
                                out=ad_t[:], in0=MT[blk][:],
                                scalar1=MTs[blk][:, k:k + 1], scalar2=0.0,
                                op0=ALU.subtract, op1=ALU.max,
                            )
                        else:
                            nc.vector.tensor_scalar(
                                out=ad_t[:], in0=MT[blk][:],
                                scalar1=MTs[blk][:, k:k + 1], scalar2=0.0,
                                op0=ALU.subtract, op1=ALU.max,
                            )
                        pair.append(ad_t)
                    ads[kk] = pair
                return ads

            def emit_banks(G, ads):
                """c-reduction + R folds for group G -> two [128,512] banks."""
                banks = []
                views = []
                for jh in range(2):
                    psD = pd.tile([128, 512], F32, tag="psD")
                    # start=True zeroes the whole bank row of every written
                    # partition, so exactly ONE start=True covering all 512
                    # cols: the -R[b,j] fold goes first.
                    nc.tensor.matmul(
                        psD[:], negR[:, 128 * jh:128 * (jh + 1)], onehotB[:],
                        start=True, stop=False,
                    )
                    for b in range(16):
                        nc.tensor.matmul(
                            psD[:, 32 * b:32 * b + 32],
                            ones1[:],
                            posRflat[:, 256 * b + GK * G:
                                      256 * b + GK * G + GK],
                            start=False, stop=False, skip_group_check=True,
                        )
                    banks.append(psD)
                    views.append(psD[:].rearrange("p (b k) -> p k b", b=16))
                # interleave jh per kk so each ad tile is consumed by its 4
                # matmuls back-to-back (keeps the ad pool shallow)
                for kk in range(GK):
                    for blk in range(2):
                        for jh in range(2):
                            outsl = views[jh][:, kk, 8 * blk:8 * blk + 8]
                            nc.tensor.matmul(
                                outsl,
                                ads[kk][blk][:, 128 * jh:128 * (jh + 1)],
                                patM,
                                start=False,
                                stop=(kk == GK - 1 and blk == 1),
                                skip_group_check=True,
                            )
                return banks

            def emit_tail(G, banks):
                """exp + S row-sum for group G."""
                for jh in range(2):
                    cm_t = cmp_.tile([128, 512], F16, tag="cm")
                    if dbg and G == 0:
                        dcp = cmp_.tile([128, 512], F32, tag="dcp")
                        nc.vector.tensor_copy(dcp[:], banks[jh][:])
                        nc.sync.dma_start(psd_o[:][jh], dcp[:])
                    nc.scalar.activation(cm_t[:], banks[jh][:], AF.Exp,
                                         scale=-1.0)
                    if dbg and G == 0:
                        dcm = cmp_.tile([128, 512], F32, tag="dcm")
                        nc.vector.tensor_copy(dcm[:], cm_t[:])
                        nc.sync.dma_start(cm_o[:][jh], dcm[:])
                    nc.tensor.matmul(
                        S_ps[:], selT[:, 32 * G + 16 * jh:32 * G + 16 * jh + 16],
                        cm_t[:],
                        start=(G == 0 and jh == 0),
                        stop=(G == NG - 1 and jh == 1),
                        skip_group_check=not (G == 0 and jh == 0),
                    )

            # ---- main loop (exp/S of group G-1 emitted after ads of G so a
            # blocked exp can't stall the ScalarE relu stream) ----
            prev = None
            for G in range(NG):
                ads = emit_ads(G)
                banks = emit_banks(G, ads)
                if prev is not None:
                    emit_tail(G - 1, prev)
                prev = banks
            emit_tail(NG - 1, prev)

            # ---- output: copy with (b,kk)->(kk,b) free permutation so the
            # DRAM DMAs are a few contiguous descriptors. Row 8 holds the
            # special k=0 sum; out row 0 comes from it via the third DMA. ----
            nc.scalar.copy(Ssb[:],
                           S_ps[:].rearrange("g (b k) -> g k b", b=16))
            dstv = out_d[:].rearrange("(g k) b -> g (k b)", g=8)
            nc.sync.dma_start(dstv[0:1, 16:512], Ssb[0:1, 16:512])
            nc.sync.dma_start(dstv[1:8], Ssb[1:8, :])
            nc.sync.dma_start(out_d[0:1, :], Ssb[8:9, 0:16])

            if dbg:
                for blk in range(2):
                    nc.sync.dma_start(mt_o[:][blk], MTs[blk][:])
                prf32 = pp.tile([1, 16 * N], F32, tag="prf32")
                nc.vector.tensor_copy(prf32[:], posRflat[:])
                nc.sync.dma_start(prf_o[:], prf32[:])
                nc.sync.dma_start(s_o[:], Ssb[:])

    nc.compile()
    return nc


def kernel(x: np.ndarray, T: np.ndarray) -> np.ndarray:
    if "nc" not in _cache:
        _cache["nc"] = build_program()
    nc = _cache["nc"]

    x = np.ascontiguousarray(x, dtype=np.float32)
    T = np.ascontiguousarray(T, dtype=np.float32)
    xT = np.ascontiguousarray(x.T.astype(np.float16))  # [A, N]

    in_maps = []
    for c in range(NCORES):
        tl = np.ascontiguousarray(
            T[:, c * BL:(c + 1) * BL, :].reshape(A, BC).astype(np.float16))
        in_maps.append({"xT": xT, "Tl": tl})

    res = run_bass_kernel_spmd(nc, in_maps, list(range(NCORES)))
    outs = [res.results[c]["out"] for c in range(NCORES)]
    return np.concatenate(outs, axis=1)                  # [N, B]


if __name__ == "__main__":
    rng = np.random.default_rng(0)
    x = rng.standard_normal((N, A)).astype(np.float32)
    T = rng.random((A, B, C), dtype=np.float32)
    out = kernel(x, T)
    print(out.shape, out.dtype, out[:3, :3])


# revision 29
# speedup vs baseline: 1.2599x; 1.0052x over previous
"""MiniBatchDiscrimination kernel for 8 Trainium2 NeuronCores.

Problem:
  x [256, 1024] f32, T [1024, 128, 16] f32
  M = einsum('na,abc->nbc', x, T)                      [N=256, B=128, C=16]
  D[k,j,b] = sum_c |M[k,b,c] - M[j,b,c]|
  Cmat = exp(-D); S = sum_j Cmat
  out = S - Cmat[:, N-1, :]; out[0] = S[0]-Cmat[0,0]; out[N-1] = S[N-1]-Cmat[N-1,N-1]

Sharding: data-parallel over B (each core owns 16 of the 128 b-channels).
The pairwise distance is independent per b, so there is no communication.

Per-core dataflow ("transposed-D" design). |d| = 2*relu(d) - d with the
linear term folded in via R[b,k] = sum_c M[k,b,c]:
  D[k,j,b] = 2*P[k,j,b] - R[b,j] + R[b,k],  P = sum_c relu(M[j]-M[k])

  PE   : MT[bc, n] = (x @ T_loc)^T (16 accumulating matmuls)
  3-way: relu-diff tiles ad_k[bc, j] = relu(MT[:, j] - MT[:, k]) split
         across DVE tensor_scalar (4x mode), ScalarE activation(Relu),
         and Pool tensor_scalar -- the elementwise work is the wall.
  PE   : c-reduction with the relu tile as the STATIONARY operand and the
         tiny one-hot pattern as the moving tensor: out[j, (b,kk)] slices
         of a [128 j, 512 (b,kk)] PSUM bank cost only 8 columns each.
         +R[b,k] / -R[b,j] are folded by two more cheap matmuls, so the
         self column is exactly 0 and exp needs no bias.
  ScE  : Cmat tile = exp(-psD) over [128, 512] (no bias, no accum).
  PE   : row sums S[k,b] = sum_j Cmat via one-hot-column stationaries into
         one [16, 512] PSUM bank; the jh=1 stationary has row 127 zeroed,
         which folds the "exclude j=255" output quirk for every k.
  fix  : out[0] needs + (Cmat[0,255]-Cmat[0,0]); two [1,16] row extracts
         (Pool) + one DVE add into the S bank.
  out  : S -> SBUF copy with k-major free-dim permutation -> one 8-descriptor
         DMA to DRAM.
"""

import os
import sys

import numpy as np

for _p in ("/opt/trn_rl_repo", os.path.expanduser("~/.axon_site/_ro/trn_rl_repo")):
    if os.path.isdir(_p) and _p not in sys.path:
        sys.path.insert(0, _p)
        break

import concourse.bass as bass
import concourse.tile as tile
from concourse import bacc, mybir
from concourse.bass_utils import run_bass_kernel_spmd

A, B, C, N = 1024, 128, 16, 256
NCORES = 8
BL = B // NCORES          # 16 b-channels per core
BC = BL * C               # 256 (b, c) pairs per core
NG = 8                    # k-groups
GK = N // NG              # 32 k's per group
F32 = mybir.dt.float32
F16 = mybir.dt.float16
F8 = mybir.dt.float8e4
ALU = mybir.AluOpType
AF = mybir.ActivationFunctionType

# per-group engine split for the 32 (k -> 2 relu-diff ops) slots
ACT_KK = {2, 8, 15, 21, 27}
POOL_KK = {5, 11, 17, 23, 29, 31}

_cache = {}


def _consts():
    # patM: moving tensor of the c-reduction; patM[p, j] = 2*(p//16 == j)
    patM = np.zeros((128, 8), np.float32)
    for p in range(128):
        patM[p, p // 16] = 2.0
    # patA1/patB1: R row-sum patterns (psR[b, j] = sum_c M[j, 16b+c])
    patA1 = np.zeros((128, 16), np.float32)
    patB1 = np.zeros((128, 16), np.float32)
    for p in range(128):
        patA1[p, p // 16] = 1.0
        patB1[p, 8 + p // 16] = 1.0
    # selT: S-accumulation stationaries; col 16*(2g+jh)+m = (m==g), with
    # row 127 zeroed for jh=1 (drops j=255 from every row sum). Row m=8 of
    # the g=0 stationaries is the special k=0 sum: drop j=0, keep j=255.
    selT = np.zeros((128, 16 * 2 * NG), np.float32)
    for g in range(NG):
        for jh in range(2):
            col = 16 * (2 * g + jh) + g
            selT[:, col] = 1.0
            if jh == 1:
                selT[127, col] = 0.0
    selT[1:, 16 * 0 + 8] = 1.0    # g=0, jh=0: j in [1, 128)
    selT[:, 16 * 1 + 8] = 1.0     # g=0, jh=1: j in [128, 256] all
    # onehotB: -R[b,j] fold moving tensor; onehotB[b', 32b+kk] = (b==b')
    onehotB = np.zeros((16, 512), np.float32)
    for b in range(16):
        onehotB[b, 32 * b:32 * b + 32] = 1.0
    return patM, patA1, patB1, selT, onehotB


def build_program(dbg: bool = False, dbg_parts: int = 7):
    # dbg_parts bitmask: 1=pattern matmuls, 2=+R_k fold, 4=-R_j fold
    nc = bacc.Bacc(
        "TRN2", target_bir_lowering=False, debug=False, enable_asserts=True
    )

    xT_d = nc.dram_tensor("xT", [A, N], F8, kind="ExternalInput")
    tl_d = nc.dram_tensor("Tl", [A, BC], F8, kind="ExternalInput")
    out_d = nc.dram_tensor("out", [N, BL], F32, kind="ExternalOutput")
    if dbg:
        mt_o = nc.dram_tensor("mt_o", [2, 128, N], F32, kind="ExternalOutput")
        prf_o = nc.dram_tensor("prf_o", [1, 16 * N], F32, kind="ExternalOutput")
        psd_o = nc.dram_tensor("psd_o", [2, 128, 512], F32,
                               kind="ExternalOutput")
        cm_o = nc.dram_tensor("cm_o", [2, 128, 512], F32,
                              kind="ExternalOutput")
        s_o = nc.dram_tensor("s_o", [16, 512], F32, kind="ExternalOutput")

    patM_np, patA1_np, patB1_np, selT_np, onehotB_np = _consts()
    cA_np = np.concatenate([patM_np, patA1_np, patB1_np, selT_np], axis=1)
    cA_d = nc.inline_tensor(cA_np.astype(np.float16), name="cA")
    cB_d = nc.inline_tensor(onehotB_np.astype(np.float16), name="cB")

    xT_v = xT_d[:].rearrange("(a p) n -> p a n", p=128)
    tl_v = tl_d[:].rearrange("(a p) m -> p a m", p=128)

    with tile.TileContext(nc) as tc:
        with (
            tc.tile_pool(name="persist", bufs=1) as pp,
            tc.tile_pool(name="ad", bufs=64) as adp,
            tc.tile_pool(name="cm", bufs=3) as cmp_,
        ):
            # ---- PE clock warmup (HAM gate holds PE at low clock for
            # ~3.5us; burn the input-DMA window on dummy matmuls) ----
            warm_t = pp.tile([128, 128], F16, tag="warm")
            nc.vector.memset(warm_t[:], 0.0)
            ones1 = pp.tile([1, 128], F16, tag="ones1")
            nc.vector.memset(ones1[:], 1.0)
            pwm_ctx = tc.tile_pool(name="psum_warm", bufs=1, space="PSUM")
            pwm = pwm_ctx.__enter__()
            wps = pwm.tile([128, 128], F32, tag="wps")
            for _ in range(32):
                nc.tensor.matmul(wps[:], warm_t[:], warm_t[:],
                                 start=True, stop=True)
            pwm_ctx.__exit__(None, None, None)

            # ---- load inputs (split across the SP and ACT HWDGE rings) ----
            xbig = pp.tile([128, 8 * N], F8, tag="xbig")
            tbig = pp.tile([128, 8 * BC], F8, tag="tbig")
            xbv = xbig[:].rearrange("p (a n) -> p a n", a=8)
            tbv = tbig[:].rearrange("p (a m) -> p a m", a=8)
            nc.sync.dma_start(xbv[:, 0:1], xT_v[:, 0:1])
            nc.scalar.dma_start(tbv[:, 0:1], tl_v[:, 0:1])
            nc.sync.dma_start(xbv[:, 1:4], xT_v[:, 1:4])
            nc.scalar.dma_start(tbv[:, 1:4], tl_v[:, 1:4])
            nc.scalar.dma_start(xbv[:, 4:8], xT_v[:, 4:8])
            nc.sync.dma_start(tbv[:, 4:8], tl_v[:, 4:8])
            xts = [xbig[:, a * N:(a + 1) * N] for a in range(8)]
            tls = [tbig[:, a * BC:(a + 1) * BC] for a in range(8)]

            cA_t = pp.tile([128, 8 + 16 + 16 + 32 * NG], F16, tag="cA")
            nc.sync.dma_start(cA_t[:], cA_d[:])
            patM = cA_t[:, 0:8]
            patA1 = cA_t[:, 8:24]
            patB1 = cA_t[:, 24:40]
            selT = cA_t[:, 40:40 + 32 * NG]
            onehotB = pp.tile([16, 512], F16, tag="cB")
            nc.scalar.dma_start(onehotB[:], cB_d[:])

            # ---- GEMM: MT[bc, n] = sum_a Tl[a, bc] * x[n, a] ----
            pmt_ctx = tc.tile_pool(name="psum_mt", bufs=2, space="PSUM")
            pmt = pmt_ctx.__enter__()
            MT = []        # f16 stream (relu-diff input, PE stationaries)
            MTs = []       # f32 scalar source for tensor_scalar scalar1
            negMT = []     # f32 bias source for ScalarE Relu
            for blk in range(2):
                ps = pmt.tile([128, N], F32, tag="psmt")
                for a in range(8):
                    nc.tensor.matmul(
                        ps[:],
                        tls[a][:, blk * 128:(blk + 1) * 128],
                        xts[a],
                        start=(a == 0),
                        stop=(a == 7),
                    )
                # mts/nmt must be exact f32 images of the f16 mt values so
                # the j==k relu-diff cancels bit-exactly
                mt_t = pp.tile([128, N], F16, tag=f"mt{blk}")
                nc.scalar.copy(mt_t[:], ps[:])
                mts_t = pp.tile([128, N], F32, tag=f"mts{blk}")
                nc.vector.tensor_copy(mts_t[:], mt_t[:])
                nmt_t = pp.tile([128, N], F32, tag=f"nmt{blk}")
                nc.vector.tensor_scalar(
                    out=nmt_t[:], in0=mts_t[:], scalar1=-1.0, scalar2=None,
                    op0=ALU.mult,
                )
                MT.append(mt_t)
                MTs.append(mts_t)
                negMT.append(nmt_t)

            # ---- R[b, j] = sum_c M[j, b, c] ----
            psR = pmt.tile([16, N], F32, tag="psmt")
            nc.tensor.matmul(psR[:], patA1, MT[0][:], start=True, stop=False)
            nc.tensor.matmul(psR[:], patB1, MT[1][:], start=False, stop=True)
            # posR/negR in f16: the +R_k and -R_j folds add exactly these
            # values, so the j==k column cancels bit-exactly.
            posR = pp.tile([16, N], F16, tag="posR")
            nc.scalar.copy(posR[:], psR[:])
            negR = pp.tile([16, N], F16, tag="negR")
            nc.scalar.mul(negR[:], psR[:], -1.0)
            pmt_ctx.__exit__(None, None, None)

            # posRflat[0, 256*b + k] = R[b, k]  (moving tensor of +R_k fold
            # must start at partition 0; one 16-descriptor SBUF->SBUF DMA)
            posRflat = pp.tile([1, 16 * N], F16, tag="posRflat")
            dstf = posRflat[:].rearrange("p (b k) -> p b k", b=16)
            nc.sync.dma_start(dstf, posR[:])

            # PSUM pools for the main loop, opened after the GEMM pools have
            # released their banks: 3 x 2-bank D tiles + 1 S bank = 7 of 8
            pd_ctx = tc.tile_pool(name="psum_d", bufs=3, space="PSUM")
            pd = pd_ctx.__enter__()
            psS_ctx = tc.tile_pool(name="psum_s", bufs=1, space="PSUM")
            psS = psS_ctx.__enter__()

            # ---- persistent result tiles ----
            S_ps = psS.tile([16, 512], F32, tag="S")
            Ssb = pp.tile([16, 512], F32, tag="Ssb")

            def emit_ads(G):
                """relu-diff tiles for group G, split across engines."""
                ads = {}
                for kk in range(GK):
                    k = GK * G + kk
                    pair = []
                    for blk in range(2):
                        ad_t = adp.tile([128, N], F16, tag="ad")
                        if kk in ACT_KK:
                            nc.scalar.activation(
                                ad_t[:], MT[blk][:], AF.Relu,
                                bias=negMT[blk][:, k:k + 1], scale=1.0,
                            )
                        elif kk in POOL_KK:
                            nc.gpsimd.tensor_scalar(
                                out=ad_t[:], in0=MT[blk][:],
                                scalar1=MTs[blk][:, k:k + 1], scalar2=0.0,
                                op0=ALU.subtract, op1=ALU.max,
                            )
                        else:
                            nc.vector.tensor_scalar(
                                out=ad_t[:], in0=MT[blk][:],
                                scalar1=MTs[blk][:, k:k + 1], scalar2=0.0,
                                op0=ALU.subtract, op1=ALU.max,
                            )
                        pair.append(ad_t)
                    ads[kk] = pair
                return ads

            def emit_banks(G, ads):
                """c-reduction + R folds for group G -> one [128,1024]
                two-bank tile (cols 512*jh + 32b + kk)."""
                psD = pd.tile([128, 1024], F32, tag="psD")
                views = []
                for jh in range(2):
                    half = psD[:, 512 * jh:512 * (jh + 1)]
                    # start=True zeroes the whole bank row of every written
                    # partition, so exactly ONE start=True covering all 512
                    # cols of the half: the -R[b,j] fold goes first.
                    nc.tensor.matmul(
                        half, negR[:, 128 * jh:128 * (jh + 1)], onehotB[:],
                        start=True, stop=False,
                        skip_group_check=(jh == 1),
                    )
                    for b in range(16):
                        nc.tensor.matmul(
                            half[:, 32 * b:32 * b + 32],
                            ones1[:],
                            posRflat[:, 256 * b + GK * G:
                                      256 * b + GK * G + GK],
                            start=False, stop=False, skip_group_check=True,
                        )
                    views.append(half.rearrange("p (b k) -> p k b", b=16))
                # interleave jh per kk so each ad tile is consumed by its 4
                # matmuls back-to-back (keeps the ad pool shallow)
                for kk in range(GK):
                    for blk in range(2):
                        for jh in range(2):
                            outsl = views[jh][:, kk, 8 * blk:8 * blk + 8]
                            nc.tensor.matmul(
                                outsl,
                                ads[kk][blk][:, 128 * jh:128 * (jh + 1)],
                                patM,
                                start=False,
                                stop=(kk == GK - 1 and blk == 1),
                                skip_group_check=True,
                            )
                return psD
                return banks

            def emit_tail(G, psD):
                """exp + S row-sum for group G."""
                cm_t = cmp_.tile([128, 1024], F16, tag="cm")
                if dbg and G == 0:
                    dcp = cmp_.tile([128, 1024], F32, tag="dcp")
                    nc.vector.tensor_copy(dcp[:], psD[:])
                    for jh in range(2):
                        nc.sync.dma_start(psd_o[:][jh],
                                          dcp[:, 512 * jh:512 * (jh + 1)])
                nc.scalar.activation(cm_t[:], psD[:], AF.Exp, scale=-1.0)
                if dbg and G == 0:
                    dcm = cmp_.tile([128, 1024], F32, tag="dcm")
                    nc.vector.tensor_copy(dcm[:], cm_t[:])
                    for jh in range(2):
                        nc.sync.dma_start(cm_o[:][jh],
                                          dcm[:, 512 * jh:512 * (jh + 1)])
                for jh in range(2):
                    nc.tensor.matmul(
                        S_ps[:], selT[:, 32 * G + 16 * jh:32 * G + 16 * jh + 16],
                        cm_t[:, 512 * jh:512 * (jh + 1)],
                        start=(G == 0 and jh == 0),
                        stop=(G == NG - 1 and jh == 1),
                        skip_group_check=not (G == 0 and jh == 0),
                    )

            # ---- main loop (exp/S of group G-1 emitted after ads of G so a
            # blocked exp can't stall the ScalarE relu stream) ----
            prev = None
            for G in range(NG):
                ads = emit_ads(G)
                banks = emit_banks(G, ads)
                if prev is not None:
                    emit_tail(G - 1, prev)
                prev = banks
            emit_tail(NG - 1, prev)

            # ---- output: copy with (b,kk)->(kk,b) free permutation so the
            # DRAM DMAs are a few contiguous descriptors. Row 8 holds the
            # special k=0 sum; out row 0 comes from it via the third DMA. ----
            nc.scalar.copy(Ssb[:],
                           S_ps[:].rearrange("g (b k) -> g k b", b=16))
            dstv = out_d[:].rearrange("(g k) b -> g (k b)", g=8)
            nc.sync.dma_start(dstv[0:1, 16:512], Ssb[0:1, 16:512])
            nc.sync.dma_start(dstv[1:8], Ssb[1:8, :])
            nc.sync.dma_start(out_d[0:1, :], Ssb[8:9, 0:16])

            if dbg:
                for blk in range(2):
                    nc.sync.dma_start(mt_o[:][blk], MTs[blk][:])
                prf32 = pp.tile([1, 16 * N], F32, tag="prf32")
                nc.vector.tensor_copy(prf32[:], posRflat[:])
                nc.sync.dma_start(prf_o[:], prf32[:])
                nc.sync.dma_start(s_o[:], Ssb[:])

            psS_ctx.__exit__(None, None, None)
            pd_ctx.__exit__(None, None, None)

    nc.compile()
    return nc


def kernel(x: np.ndarray, T: np.ndarray) -> np.ndarray:
    if "nc" not in _cache:
        _cache["nc"] = build_program()
    nc = _cache["nc"]

    np_f8 = mybir.dt.np(F8)
    x = np.ascontiguousarray(x, dtype=np.float32)
    T = np.ascontiguousarray(T, dtype=np.float32)
    xT = np.ascontiguousarray(x.T.astype(np_f8))       # [A, N]

    in_maps = []
    for c in range(NCORES):
        tl = np.ascontiguousarray(
            T[:, c * BL:(c + 1) * BL, :].reshape(A, BC).astype(np_f8))
        in_maps.append({"xT": xT, "Tl": tl})

    res = run_bass_kernel_spmd(nc, in_maps, list(range(NCORES)))
    outs = [res.results[c]["out"] for c in range(NCORES)]
    return np.concatenate(outs, axis=1)                  # [N, B]


if __name__ == "__main__":
    rng = np.random.default_rng(0)
    x = rng.standard_normal((N, A)).astype(np.float32)
    T = rng.random((A, B, C), dtype=np.float32)
    out = kernel(x, T)
    print(out.shape, out.dtype, out[:3, :3])


# revision 31
# speedup vs baseline: 1.2675x; 1.0060x over previous
"""MiniBatchDiscrimination kernel for 8 Trainium2 NeuronCores.

Problem:
  x [256, 1024] f32, T [1024, 128, 16] f32
  M = einsum('na,abc->nbc', x, T)                      [N=256, B=128, C=16]
  D[k,j,b] = sum_c |M[k,b,c] - M[j,b,c]|
  Cmat = exp(-D); S = sum_j Cmat
  out = S - Cmat[:, N-1, :]; out[0] = S[0]-Cmat[0,0]; out[N-1] = S[N-1]-Cmat[N-1,N-1]

Sharding: data-parallel over B (each core owns 16 of the 128 b-channels).
The pairwise distance is independent per b, so there is no communication.

Per-core dataflow ("transposed-D" design). |d| = 2*relu(d) - d with the
linear term folded in via R[b,k] = sum_c M[k,b,c]:
  D[k,j,b] = 2*P[k,j,b] - R[b,j] + R[b,k],  P = sum_c relu(M[j]-M[k])

  PE   : MT[bc, n] = (x @ T_loc)^T (16 accumulating matmuls)
  3-way: relu-diff tiles ad_k[bc, j] = relu(MT[:, j] - MT[:, k]) split
         across DVE tensor_scalar (4x mode), ScalarE activation(Relu),
         and Pool tensor_scalar -- the elementwise work is the wall.
  PE   : c-reduction with the relu tile as the STATIONARY operand and the
         tiny one-hot pattern as the moving tensor: out[j, (b,kk)] slices
         of a [128 j, 512 (b,kk)] PSUM bank cost only 8 columns each.
         +R[b,k] / -R[b,j] are folded by two more cheap matmuls, so the
         self column is exactly 0 and exp needs no bias.
  ScE  : Cmat tile = exp(-psD) over [128, 512] (no bias, no accum).
  PE   : row sums S[k,b] = sum_j Cmat via one-hot-column stationaries into
         one [16, 512] PSUM bank; the jh=1 stationary has row 127 zeroed,
         which folds the "exclude j=255" output quirk for every k.
  fix  : out[0] needs + (Cmat[0,255]-Cmat[0,0]); two [1,16] row extracts
         (Pool) + one DVE add into the S bank.
  out  : S -> SBUF copy with k-major free-dim permutation -> one 8-descriptor
         DMA to DRAM.
"""

import os
import sys

import numpy as np

for _p in ("/opt/trn_rl_repo", os.path.expanduser("~/.axon_site/_ro/trn_rl_repo")):
    if os.path.isdir(_p) and _p not in sys.path:
        sys.path.insert(0, _p)
        break

import concourse.bass as bass
import concourse.tile as tile
from concourse import bacc, mybir
from concourse.bass_utils import run_bass_kernel_spmd

A, B, C, N = 1024, 128, 16, 256
NCORES = 8
BL = B // NCORES          # 16 b-channels per core
BC = BL * C               # 256 (b, c) pairs per core
NG = 8                    # k-groups
GK = N // NG              # 32 k's per group
F32 = mybir.dt.float32
F16 = mybir.dt.float16
F8 = mybir.dt.float8e4
ALU = mybir.AluOpType
AF = mybir.ActivationFunctionType

# per-group engine split for the 32 (k -> 2 relu-diff ops) slots
ACT_KK = {2, 8, 15, 21, 27}
POOL_KK = {5, 11, 17, 23, 29, 31}

_cache = {}


def _consts():
    # patM: moving tensor of the c-reduction; patM[p, j] = 2*(p//16 == j)
    patM = np.zeros((128, 8), np.float32)
    for p in range(128):
        patM[p, p // 16] = 2.0
    # patA1/patB1: R row-sum patterns (psR[b, j] = sum_c M[j, 16b+c])
    patA1 = np.zeros((128, 16), np.float32)
    patB1 = np.zeros((128, 16), np.float32)
    for p in range(128):
        patA1[p, p // 16] = 1.0
        patB1[p, 8 + p // 16] = 1.0
    # selT: S-accumulation stationaries; col 16*(2g+jh)+m = (m==g), with
    # row 127 zeroed for jh=1 (drops j=255 from every row sum). Row m=8 of
    # the g=0 stationaries is the special k=0 sum: drop j=0, keep j=255.
    selT = np.zeros((128, 16 * 2 * NG), np.float32)
    for g in range(NG):
        for jh in range(2):
            col = 16 * (2 * g + jh) + g
            selT[:, col] = 1.0
            if jh == 1:
                selT[127, col] = 0.0
    selT[1:, 16 * 0 + 8] = 1.0    # g=0, jh=0: j in [1, 128)
    selT[:, 16 * 1 + 8] = 1.0     # g=0, jh=1: j in [128, 256] all
    # onehotB: -R[b,j] fold moving tensor; onehotB[b', 32b+kk] = (b==b')
    onehotB = np.zeros((16, 512), np.float32)
    for b in range(16):
        onehotB[b, 32 * b:32 * b + 32] = 1.0
    return patM, patA1, patB1, selT, onehotB


def build_program(dbg: bool = False, dbg_parts: int = 7):
    # dbg_parts bitmask: 1=pattern matmuls, 2=+R_k fold, 4=-R_j fold
    nc = bacc.Bacc(
        "TRN2", target_bir_lowering=False, debug=False, enable_asserts=True
    )

    xT_d = nc.dram_tensor("xT", [A, N], F8, kind="ExternalInput")
    tl_d = nc.dram_tensor("Tl", [A, BC], F8, kind="ExternalInput")
    out_d = nc.dram_tensor("out", [N, BL], F32, kind="ExternalOutput")
    if dbg:
        mt_o = nc.dram_tensor("mt_o", [2, 128, N], F32, kind="ExternalOutput")
        prf_o = nc.dram_tensor("prf_o", [1, 16 * N], F32, kind="ExternalOutput")
        psd_o = nc.dram_tensor("psd_o", [2, 128, 512], F32,
                               kind="ExternalOutput")
        cm_o = nc.dram_tensor("cm_o", [2, 128, 512], F32,
                              kind="ExternalOutput")
        s_o = nc.dram_tensor("s_o", [16, 512], F32, kind="ExternalOutput")

    patM_np, patA1_np, patB1_np, selT_np, onehotB_np = _consts()
    cA_np = np.concatenate([patM_np, patA1_np, patB1_np, selT_np], axis=1)
    cA_d = nc.inline_tensor(cA_np.astype(np.float16), name="cA")
    cB_d = nc.inline_tensor(onehotB_np.astype(np.float16), name="cB")

    xT_v = xT_d[:].rearrange("(a p) n -> p a n", p=128)
    tl_v = tl_d[:].rearrange("(a p) m -> p a m", p=128)

    with tile.TileContext(nc) as tc:
        with (
            tc.tile_pool(name="persist", bufs=1) as pp,
            tc.tile_pool(name="ad", bufs=64) as adp,
            tc.tile_pool(name="cm", bufs=3) as cmp_,
        ):
            # ---- PE clock warmup (HAM gate holds PE at low clock for
            # ~3.5us; burn the input-DMA window on dummy matmuls) ----
            warm_t = pp.tile([128, 128], F16, tag="warm")
            nc.vector.memset(warm_t[:], 0.0)
            ones1 = pp.tile([1, 128], F16, tag="ones1")
            nc.vector.memset(ones1[:], 1.0)
            pwm_ctx = tc.tile_pool(name="psum_warm", bufs=1, space="PSUM")
            pwm = pwm_ctx.__enter__()
            wps = pwm.tile([128, 128], F32, tag="wps")
            for _ in range(32):
                nc.tensor.matmul(wps[:], warm_t[:], warm_t[:],
                                 start=True, stop=True)
            pwm_ctx.__exit__(None, None, None)

            # ---- load inputs (split across the SP and ACT HWDGE rings) ----
            xbig = pp.tile([128, 8 * N], F8, tag="xbig")
            tbig = pp.tile([128, 8 * BC], F8, tag="tbig")
            xbv = xbig[:].rearrange("p (a n) -> p a n", a=8)
            tbv = tbig[:].rearrange("p (a m) -> p a m", a=8)
            nc.sync.dma_start(xbv[:, 0:2], xT_v[:, 0:2])
            nc.scalar.dma_start(tbv[:, 0:2], tl_v[:, 0:2])
            nc.sync.dma_start(xbv[:, 2:8], xT_v[:, 2:8])
            nc.scalar.dma_start(tbv[:, 2:8], tl_v[:, 2:8])
            xts = [xbig[:, a * N:(a + 1) * N] for a in range(8)]
            tls = [tbig[:, a * BC:(a + 1) * BC] for a in range(8)]

            cA_t = pp.tile([128, 8 + 16 + 16 + 32 * NG], F16, tag="cA")
            nc.sync.dma_start(cA_t[:], cA_d[:])
            patM = cA_t[:, 0:8]
            patA1 = cA_t[:, 8:24]
            patB1 = cA_t[:, 24:40]
            selT = cA_t[:, 40:40 + 32 * NG]
            onehotB = pp.tile([16, 512], F16, tag="cB")
            nc.scalar.dma_start(onehotB[:], cB_d[:])

            # ---- GEMM: MT[bc, n] = sum_a Tl[a, bc] * x[n, a] ----
            pmt_ctx = tc.tile_pool(name="psum_mt", bufs=2, space="PSUM")
            pmt = pmt_ctx.__enter__()
            MT = []        # f16 stream (relu-diff input, PE stationaries)
            MTs = []       # f32 scalar source for tensor_scalar scalar1
            negMT = []     # f32 bias source for ScalarE Relu
            for blk in range(2):
                ps = pmt.tile([128, N], F32, tag="psmt")
                for a in range(8):
                    nc.tensor.matmul(
                        ps[:],
                        tls[a][:, blk * 128:(blk + 1) * 128],
                        xts[a],
                        start=(a == 0),
                        stop=(a == 7),
                    )
                # mts/nmt must be exact f32 images of the f16 mt values so
                # the j==k relu-diff cancels bit-exactly
                mt_t = pp.tile([128, N], F16, tag=f"mt{blk}")
                nc.scalar.copy(mt_t[:], ps[:])
                mts_t = pp.tile([128, N], F32, tag=f"mts{blk}")
                nc.vector.tensor_copy(mts_t[:], mt_t[:])
                nmt_t = pp.tile([128, N], F32, tag=f"nmt{blk}")
                nc.vector.tensor_scalar(
                    out=nmt_t[:], in0=mts_t[:], scalar1=-1.0, scalar2=None,
                    op0=ALU.mult,
                )
                MT.append(mt_t)
                MTs.append(mts_t)
                negMT.append(nmt_t)

            # ---- R[b, j] = sum_c M[j, b, c] ----
            psR = pmt.tile([16, N], F32, tag="psmt")
            nc.tensor.matmul(psR[:], patA1, MT[0][:], start=True, stop=False)
            nc.tensor.matmul(psR[:], patB1, MT[1][:], start=False, stop=True)
            # posR/negR in f16: the +R_k and -R_j folds add exactly these
            # values, so the j==k column cancels bit-exactly.
            posR = pp.tile([16, N], F16, tag="posR")
            nc.scalar.copy(posR[:], psR[:])
            negR = pp.tile([16, N], F16, tag="negR")
            nc.scalar.mul(negR[:], psR[:], -1.0)
            pmt_ctx.__exit__(None, None, None)

            # posRflat[0, 256*b + k] = R[b, k]  (moving tensor of +R_k fold
            # must start at partition 0; one 16-descriptor SBUF->SBUF DMA)
            posRflat = pp.tile([1, 16 * N], F16, tag="posRflat")
            dstf = posRflat[:].rearrange("p (b k) -> p b k", b=16)
            nc.sync.dma_start(dstf, posR[:])

            # PSUM pools for the main loop, opened after the GEMM pools have
            # released their banks: 3 x 2-bank D tiles + 1 S bank = 7 of 8
            pd_ctx = tc.tile_pool(name="psum_d", bufs=3, space="PSUM")
            pd = pd_ctx.__enter__()
            psS_ctx = tc.tile_pool(name="psum_s", bufs=1, space="PSUM")
            psS = psS_ctx.__enter__()

            # ---- persistent result tiles ----
            S_ps = psS.tile([16, 512], F32, tag="S")
            Ssb = pp.tile([16, 512], F32, tag="Ssb")

            def emit_ads(G):
                """relu-diff tiles for group G, split across engines."""
                ads = {}
                for kk in range(GK):
                    k = GK * G + kk
                    pair = []
                    for blk in range(2):
                        ad_t = adp.tile([128, N], F16, tag="ad")
                        if kk in ACT_KK:
                            nc.scalar.activation(
                                ad_t[:], MT[blk][:], AF.Relu,
                                bias=negMT[blk][:, k:k + 1], scale=1.0,
                            )
                        elif kk in POOL_KK:
                            nc.gpsimd.tensor_scalar(
                                out=ad_t[:], in0=MT[blk][:],
                                scalar1=MTs[blk][:, k:k + 1], scalar2=0.0,
                                op0=ALU.subtract, op1=ALU.max,
                            )
                        else:
                            nc.vector.tensor_scalar(
                                out=ad_t[:], in0=MT[blk][:],
                                scalar1=MTs[blk][:, k:k + 1], scalar2=0.0,
                                op0=ALU.subtract, op1=ALU.max,
                            )
                        pair.append(ad_t)
                    ads[kk] = pair
                return ads

            def emit_banks(G, ads):
                """c-reduction + R folds for group G -> one [128,1024]
                two-bank tile (cols 512*jh + 32b + kk)."""
                psD = pd.tile([128, 1024], F32, tag="psD")
                views = []
                for jh in range(2):
                    half = psD[:, 512 * jh:512 * (jh + 1)]
                    # start=True zeroes the whole bank row of every written
                    # partition, so exactly ONE start=True covering all 512
                    # cols of the half: the -R[b,j] fold goes first.
                    nc.tensor.matmul(
                        half, negR[:, 128 * jh:128 * (jh + 1)], onehotB[:],
                        start=True, stop=False,
                        skip_group_check=(jh == 1),
                    )
                    for b in range(16):
                        nc.tensor.matmul(
                            half[:, 32 * b:32 * b + 32],
                            ones1[:],
                            posRflat[:, 256 * b + GK * G:
                                      256 * b + GK * G + GK],
                            start=False, stop=False, skip_group_check=True,
                        )
                    views.append(half.rearrange("p (b k) -> p k b", b=16))
                # interleave jh per kk so each ad tile is consumed by its 4
                # matmuls back-to-back (keeps the ad pool shallow)
                for kk in range(GK):
                    for blk in range(2):
                        for jh in range(2):
                            outsl = views[jh][:, kk, 8 * blk:8 * blk + 8]
                            nc.tensor.matmul(
                                outsl,
                                ads[kk][blk][:, 128 * jh:128 * (jh + 1)],
                                patM,
                                start=False,
                                stop=(kk == GK - 1 and blk == 1),
                                skip_group_check=True,
                            )
                return psD
                return banks

            def emit_tail(G, psD):
                """exp + S row-sum for group G."""
                cm_t = cmp_.tile([128, 1024], F16, tag="cm")
                if dbg and G == 0:
                    dcp = cmp_.tile([128, 1024], F32, tag="dcp")
                    nc.vector.tensor_copy(dcp[:], psD[:])
                    for jh in range(2):
                        nc.sync.dma_start(psd_o[:][jh],
                                          dcp[:, 512 * jh:512 * (jh + 1)])
                nc.scalar.activation(cm_t[:], psD[:], AF.Exp, scale=-1.0)
                if dbg and G == 0:
                    dcm = cmp_.tile([128, 1024], F32, tag="dcm")
                    nc.vector.tensor_copy(dcm[:], cm_t[:])
                    for jh in range(2):
                        nc.sync.dma_start(cm_o[:][jh],
                                          dcm[:, 512 * jh:512 * (jh + 1)])
                for jh in range(2):
                    nc.tensor.matmul(
                        S_ps[:], selT[:, 32 * G + 16 * jh:32 * G + 16 * jh + 16],
                        cm_t[:, 512 * jh:512 * (jh + 1)],
                        start=(G == 0 and jh == 0),
                        stop=(G == NG - 1 and jh == 1),
                        skip_group_check=not (G == 0 and jh == 0),
                    )

            # ---- main loop: exp(G-1) first (its bank closed at the end of
            # the previous iteration; freeing it early keeps the PSUM ring
            # loose), then ads/banks of G ----
            prev = None
            for G in range(NG):
                if prev is not None:
                    emit_tail(G - 1, prev)
                ads = emit_ads(G)
                banks = emit_banks(G, ads)
                prev = banks
            emit_tail(NG - 1, prev)

            # ---- output: copy with (b,kk)->(kk,b) free permutation so the
            # DRAM DMAs are a few contiguous descriptors. Row 8 holds the
            # special k=0 sum; out row 0 comes from it via the third DMA. ----
            nc.scalar.copy(Ssb[:],
                           S_ps[:].rearrange("g (b k) -> g k b", b=16))
            dstv = out_d[:].rearrange("(g k) b -> g (k b)", g=8)
            nc.sync.dma_start(dstv[0:1, 16:512], Ssb[0:1, 16:512])
            nc.sync.dma_start(dstv[1:8], Ssb[1:8, :])
            nc.sync.dma_start(out_d[0:1, :], Ssb[8:9, 0:16])

            if dbg:
                for blk in range(2):
                    nc.sync.dma_start(mt_o[:][blk], MTs[blk][:])
                prf32 = pp.tile([1, 16 * N], F32, tag="prf32")
                nc.vector.tensor_copy(prf32[:], posRflat[:])
                nc.sync.dma_start(prf_o[:], prf32[:])
                nc.sync.dma_start(s_o[:], Ssb[:])

            psS_ctx.__exit__(None, None, None)
            pd_ctx.__exit__(None, None, None)

    nc.compile()
    return nc


def kernel(x: np.ndarray, T: np.ndarray) -> np.ndarray:
    if "nc" not in _cache:
        _cache["nc"] = build_program()
    nc = _cache["nc"]

    np_f8 = mybir.dt.np(F8)
    x = np.ascontiguousarray(x, dtype=np.float32)
    T = np.ascontiguousarray(T, dtype=np.float32)
    xT = np.ascontiguousarray(x.T.astype(np_f8))       # [A, N]

    in_maps = []
    for c in range(NCORES):
        tl = np.ascontiguousarray(
            T[:, c * BL:(c + 1) * BL, :].reshape(A, BC).astype(np_f8))
        in_maps.append({"xT": xT, "Tl": tl})

    res = run_bass_kernel_spmd(nc, in_maps, list(range(NCORES)))
    outs = [res.results[c]["out"] for c in range(NCORES)]
    return np.concatenate(outs, axis=1)                  # [N, B]


if __name__ == "__main__":
    rng = np.random.default_rng(0)
    x = rng.standard_normal((N, A)).astype(np.float32)
    T = rng.random((A, B, C), dtype=np.float32)
    out = kernel(x, T)
    print(out.shape, out.dtype, out[:3, :3])
